# revision 4
# baseline (speedup 1.0000x reference)
"""Additive-attention layer (Bahdanau-style) on 8 TRN2 NeuronCores.

Reference computation (per batch b):
    q_proj = query @ Wa_w.T + Wa_b                      # [1, H]
    k_proj = keys  @ Ua_w.T + Ua_b                      # [S, H]
    e      = tanh(q_proj + k_proj)                      # [S, H]
    scores = e @ Va_w.T (+ Va_b)                        # [S]  (Va_b dropped:
                                                        #  softmax shift-invariant)
    weights = softmax(scores)                           # [S]
    context = weights @ keys                            # [H]
    returns (context [B,1,H], weights [B,1,S])

Sharding: data-parallel over batch B=32 -> 4 batches per core; the small
Wa/Ua/Va weights are replicated. No collectives needed; the host
concatenates per-core outputs.

Device-side layout choices:
  - keys are shipped pre-transposed per batch as keysT [H, S] (bf16), so the
    TensorEngine can contract over H without any on-device transposes, and
    the context reduction over S runs on the VectorEngine with keysT resident.
  - Ua_w/Wa_w are shipped transposed ([h, o]) so they slice directly into
    matmul lhsT operands.
  - e is produced transposed (eT [o, s]) so the scores reduction over o is a
    TensorEngine matmul against Va and softmax lands in [1, S] layout on a
    single partition.
  - q_proj is folded into the tanh as a per-partition activation bias.

bf16 is used for the big matmul inputs (keys, Ua, Va, e); accumulation is
fp32 in PSUM. Measured end-to-end relative error vs the fp32 reference is
~2e-3.

This file is self-contained: it builds the Bass graph on first call and
runs it via run_bass_kernel_spmd on cores 0-7.
"""

import os
import numpy as np
import ml_dtypes

import concourse.bass as bass
import concourse.mybir as mybir
from concourse.tile import TileContext
from concourse.bass_utils import run_bass_kernel_spmd

B, S, H = 32, 2048, 1024
NCORES = 8
BL = B // NCORES          # batches per core = 4
P = 128                   # partitions
HJ = H // P               # h-chunks = 8
OC = H // P               # o-chunks = 8
SBW = 512                 # s-block width (PSUM bank = 512 fp32)
SB = S // SBW             # s-blocks = 4

F32 = mybir.dt.float32
BF16 = mybir.dt.bfloat16
KDT = BF16                # dtype of keysT/Ua/Va/e on device
KDT_NP = ml_dtypes.bfloat16

TANH = mybir.ActivationFunctionType.Tanh
EXP = mybir.ActivationFunctionType.Exp
MULT = mybir.AluOpType.mult
AX_X = mybir.AxisListType.X


def _split_multi_waits(nc):
    """This container's walrus rejects >1 sync-wait per instruction. Hoist
    extra waits onto NoOps inserted just before, on the same engine (engines
    run their stream in order, so happens-before edges are preserved)."""
    uid = 0
    for f in nc.m.functions:
        for bb in f.blocks:
            out = []
            changed = False
            for inst in bb.instructions:
                si = inst.sync_info
                waits = list(si.on_wait) if (si is not None and si.on_wait) else []
                if len(waits) > 1:
                    changed = True
                    for w in waits[:-1]:
                        uid += 1
                        nop = mybir.InstNoOp(name=f"waitsplit_{uid}", ins=[], outs=[])
                        nop.engine = inst.engine
                        nop.sync_info = mybir.SyncInfo(on_update=[], on_wait=[w])
                        out.append(nop)
                    si.on_wait = [waits[-1]]
                out.append(inst)
            if changed:
                bb.instructions = out
    return nc


def _build():
    nc = bass.Bass("TRN2", target_bir_lowering=False, debug=False, num_devices=NCORES)

    keysT = nc.declare_dram_parameter("keysT", [BL, H, S], KDT, isOutput=False)
    UaT = nc.declare_dram_parameter("UaT", [H, H], KDT, isOutput=False)
    WaT = nc.declare_dram_parameter("WaT", [H, H], F32, isOutput=False)
    qT = nc.declare_dram_parameter("qT", [H, BL], F32, isOutput=False)
    bsumT = nc.declare_dram_parameter("bsumT", [P, HJ], F32, isOutput=False)
    vaT = nc.declare_dram_parameter("vaT", [P, OC], KDT, isOutput=False)
    out = nc.declare_dram_parameter("out", [BL, H + S], F32, isOutput=True)

    with TileContext(nc) as tc:
        with (
            tc.tile_pool(name="weights", bufs=1) as wpool,
            tc.tile_pool(name="keys", bufs=2) as kpool,
            tc.tile_pool(name="work", bufs=3) as wkpool,
            tc.tile_pool(name="small", bufs=1) as smpool,
            tc.tile_pool(name="dram", bufs=2, space="DRAM") as dpool,
            tc.tile_pool(name="psum", bufs=4, space="PSUM") as ppool,
        ):
            # ---- persistent weights/constants ----
            ua_sb = wpool.tile([P, HJ * H], KDT)       # [h | hj*H + o]
            for hj in range(HJ):
                nc.sync.dma_start(ua_sb[:, hj * H:(hj + 1) * H],
                                  UaT[hj * P:(hj + 1) * P, :])
            wa_sb = wpool.tile([P, HJ * H], F32)
            for hj in range(HJ):
                nc.sync.dma_start(wa_sb[:, hj * H:(hj + 1) * H],
                                  WaT[hj * P:(hj + 1) * P, :])
            qt_sb = smpool.tile([P, HJ * BL], F32)     # [h | hj*BL + b]
            for hj in range(HJ):
                nc.sync.dma_start(qt_sb[:, hj * BL:(hj + 1) * BL],
                                  qT[hj * P:(hj + 1) * P, :])
            bs_sb = smpool.tile([P, HJ], F32)
            nc.sync.dma_start(bs_sb[:], bsumT[:, :])
            va_sb = smpool.tile([P, OC], KDT)
            nc.sync.dma_start(va_sb[:], vaT[:, :])

            qbT = smpool.tile([P, OC * BL], F32)       # [o | oc*BL + b]
            ctxT = smpool.tile([P, BL * HJ], F32)      # [h | b*HJ + hj]

            # ---- q_proj (transposed): qbT[o, b] = (Wa @ q)[o] + Wa_b + Ua_b ----
            for oc in range(OC):
                pq = ppool.tile([P, BL], F32, tag="sc", name=f"pq_{oc}")
                for hj in range(HJ):
                    nc.tensor.matmul(
                        pq[:],
                        wa_sb[:, hj * H + oc * P: hj * H + (oc + 1) * P],
                        qt_sb[:, hj * BL:(hj + 1) * BL],
                        start=(hj == 0), stop=(hj == HJ - 1),
                    )
                nc.vector.tensor_scalar_add(
                    qbT[:, oc * BL:(oc + 1) * BL], pq[:], bs_sb[:, oc:oc + 1])

            for b in range(BL):
                # ---- per-batch keysT, resident in SBUF (bf16, 4 MB) ----
                kt = kpool.tile([P, HJ * S], KDT, tag="kT", name=f"kt_{b}")
                for hj in range(HJ):
                    nc.sync.dma_start(kt[:, hj * S:(hj + 1) * S],
                                      keysT[b, hj * P:(hj + 1) * P, :])

                # ---- scores: eT = tanh(Ua@keysT + qb); sc = Va.T @ eT ----
                psc = [ppool.tile([1, SBW], F32, tag="sc", name=f"psc_{b}_{sb}")
                       for sb in range(SB)]
                for oc in range(OC):
                    pe = [ppool.tile([P, SBW], F32, tag="e", name=f"pe_{b}_{oc}_{sb}")
                          for sb in range(SB)]
                    for hj in range(HJ):
                        lhs = ua_sb[:, hj * H + oc * P: hj * H + (oc + 1) * P]
                        for sb in range(SB):
                            nc.tensor.matmul(
                                pe[sb][:], lhs,
                                kt[:, hj * S + sb * SBW: hj * S + sb * SBW + SBW],
                                start=(hj == 0), stop=(hj == HJ - 1),
                            )
                    for sb in range(SB):
                        et = wkpool.tile([P, SBW], KDT, tag="eT", bufs=6,
                                         name=f"et_{b}_{oc}_{sb}")
                        nc.scalar.activation(
                            et[:], pe[sb][:], TANH,
                            bias=qbT[:, oc * BL + b: oc * BL + b + 1], scale=1.0)
                        nc.tensor.matmul(
                            psc[sb][:], va_sb[:, oc:oc + 1], et[:],
                            start=(oc == 0), stop=(oc == OC - 1),
                            skip_group_check=True,
                        )
                scores = smpool.tile([1, S], F32, tag="scores", bufs=2,
                                     name=f"scores_{b}")
                for sb in range(SB):
                    nc.scalar.copy(scores[0:1, sb * SBW:(sb + 1) * SBW],
                                   psc[sb][:])

                # ---- softmax over [1, S] ----
                mx = smpool.tile([1, 1], F32, tag="mx", bufs=2, name=f"mx_{b}")
                nc.vector.reduce_max(mx[:], scores[0:1, :], axis=AX_X)
                nmx = smpool.tile([1, 1], F32, tag="nmx", bufs=2, name=f"nmx_{b}")
                nc.scalar.mul(nmx[:], mx[:], -1.0)
                zz = smpool.tile([1, 1], F32, tag="zz", bufs=2, name=f"zz_{b}")
                expv = smpool.tile([1, S], F32, tag="expv", bufs=1,
                                   name=f"expv_{b}")
                nc.scalar.activation(expv[0:1, :], scores[0:1, :], EXP,
                                     bias=nmx[:], scale=1.0, accum_out=zz[:])
                rz = smpool.tile([1, 1], F32, tag="rz", bufs=2, name=f"rz_{b}")
                nc.vector.reciprocal(rz[:], zz[:])
                wrow = smpool.tile([1, S], F32, tag="wrow", bufs=1,
                                   name=f"wrow_{b}")
                nc.vector.tensor_scalar_mul(wrow[0:1, :], expv[0:1, :], rz[:])
                nc.sync.dma_start(out[b, H:H + S], wrow[0:1, :])

                # ---- broadcast weights across partitions (DRAM bounce) ----
                wdram = dpool.tile([1, S], F32, tag="wd", name=f"wd_{b}")
                nc.sync.dma_start(wdram[:], wrow[0:1, :])
                wr = wkpool.tile([P, S], F32, tag="wrep", bufs=2, name=f"wr_{b}")
                nc.sync.dma_start(wr[:], wdram[0:1, :].partition_broadcast(P))

                # ---- context: ctxT[h] = sum_s keysT[h, s] * w[s] ----
                for hj in range(HJ):
                    pr = wkpool.tile([P, S], F32, tag="prod", bufs=2,
                                     name=f"pr_{b}_{hj}")
                    nc.vector.tensor_tensor(out=pr[:], in0=kt[:, hj * S:(hj + 1) * S],
                                            in1=wr[:], op=MULT)
                    nc.vector.reduce_sum(ctxT[:, b * HJ + hj: b * HJ + hj + 1],
                                         pr[:], axis=AX_X)
                nc.sync.dma_start(out[b, 0:H].rearrange("(hj p) -> p hj", p=P),
                                  ctxT[:, b * HJ:(b + 1) * HJ])

    _split_multi_waits(nc)
    return nc


_NC_CACHE = {}


def _get_nc():
    if "nc" not in _NC_CACHE:
        _NC_CACHE["nc"] = _build()
    return _NC_CACHE["nc"]


LAST_RESULTS = {}


def kernel(**inputs):
    query = np.asarray(inputs["query"], np.float32)    # [B, 1, H]
    keys = np.asarray(inputs["keys"], np.float32)      # [B, S, H]
    Wa_w = np.asarray(inputs["Wa_w"], np.float32)      # [H, H]
    Wa_b = np.asarray(inputs["Wa_b"], np.float32)      # [H]
    Ua_w = np.asarray(inputs["Ua_w"], np.float32)      # [H, H]
    Ua_b = np.asarray(inputs["Ua_b"], np.float32)      # [H]
    Va_w = np.asarray(inputs["Va_w"], np.float32)      # [1, H]
    # Va_b shifts every score equally; softmax is shift-invariant and scores
    # are not returned, so it is dropped.

    keysT = np.empty((B, H, S), dtype=KDT_NP)
    for b in range(B):
        keysT[b] = keys[b].T.astype(KDT_NP)
    UaT = np.ascontiguousarray(Ua_w.T).astype(KDT_NP)
    WaT = np.ascontiguousarray(Wa_w.T)
    bsum = Wa_b + Ua_b
    bsumT = np.ascontiguousarray(bsum.reshape(HJ, P).T)
    vaT = np.ascontiguousarray(Va_w[0].reshape(OC, P).T).astype(KDT_NP)
    queryT = np.ascontiguousarray(query[:, 0, :].T)    # [H, B]

    in_maps = []
    for c in range(NCORES):
        bsl = slice(c * BL, (c + 1) * BL)
        in_maps.append({
            "keysT": keysT[bsl],
            "UaT": UaT,
            "WaT": WaT,
            "qT": np.ascontiguousarray(queryT[:, bsl]),
            "bsumT": bsumT,
            "vaT": vaT,
        })

    nc = _get_nc()
    trace = bool(int(os.environ.get("KERNEL_TRACE", "0")))
    res = run_bass_kernel_spmd(nc, in_maps, core_ids=list(range(NCORES)),
                               trace=trace)
    LAST_RESULTS["exec_time_ns"] = res.exec_time_ns
    LAST_RESULTS["bass_results"] = res

    full = np.concatenate([np.asarray(res.results[c]["out"]) for c in range(NCORES)],
                          axis=0)                      # [B, H+S]
    context = np.ascontiguousarray(full[:, :H].reshape(B, 1, H), dtype=np.float32)
    weights = np.ascontiguousarray(full[:, H:].reshape(B, 1, S), dtype=np.float32)
    return (context, weights)


# revision 5
# speedup vs baseline: 1.1049x; 1.1049x over previous
"""Additive-attention layer (Bahdanau-style) on 8 TRN2 NeuronCores.

Reference computation (per batch b):
    q_proj = query @ Wa_w.T + Wa_b                      # [1, H]
    k_proj = keys  @ Ua_w.T + Ua_b                      # [S, H]
    e      = tanh(q_proj + k_proj)                      # [S, H]
    scores = e @ Va_w.T (+ Va_b)                        # [S]  (Va_b dropped:
                                                        #  softmax shift-invariant)
    weights = softmax(scores)                           # [S]
    context = weights @ keys                            # [H]
    returns (context [B,1,H], weights [B,1,S])

Sharding: data-parallel over batch B=32 -> 4 batches per core; the small
Wa/Ua/Va weights are replicated. No collectives; the host concatenates
per-core outputs.

Device-side layout:
  - keys ship pre-transposed per batch as keysT [H, S] (bf16): TensorEngine
    contracts over H with no on-device transposes, and the context reduction
    over S runs on the VectorEngine against the resident keysT.
  - Ua_w/Wa_w ship transposed ([h, o]) to slice directly into matmul lhsT.
  - e is produced transposed (eT [o, s]); the scores reduction over o is a
    TensorE matmul against Va, done as a dense run per batch (decoupled from
    the tanh pipeline), and softmax lands in [1, S] on one partition.
  - q_proj folds into the tanh as a per-partition activation bias.
  - weights are replicated across partitions for the context reduction via a
    TensorE ones-matmul (rank-1 broadcast), avoiding slow gather DMAs.

bf16 inputs for the big matmuls, fp32 PSUM accumulation. Measured rel err vs
the fp32 reference ~2.4e-3.
"""

import os
import numpy as np
import ml_dtypes

import concourse.bass as bass
import concourse.mybir as mybir
from concourse.tile import TileContext
from concourse.bass_utils import run_bass_kernel_spmd

B, S, H = 32, 2048, 1024
NCORES = 8
BL = B // NCORES          # batches per core = 4
P = 128                   # partitions
HJ = H // P               # h-chunks = 8
OC = H // P               # o-chunks = 8
SBW = 512                 # s-block width (PSUM bank = 512 fp32)
SB = S // SBW             # s-blocks = 4

F32 = mybir.dt.float32
BF16 = mybir.dt.bfloat16
KDT = BF16
KDT_NP = ml_dtypes.bfloat16

TANH = mybir.ActivationFunctionType.Tanh
EXP = mybir.ActivationFunctionType.Exp
MULT = mybir.AluOpType.mult
AX_X = mybir.AxisListType.X

N_WARMUP = 36             # PE warmup matmuls (cover the HAM cold window)


def _split_multi_waits(nc):
    """This container's walrus rejects >1 sync-wait per instruction. Hoist
    extra waits onto NoOps inserted just before, on the same engine (engines
    run their stream in order, so happens-before edges are preserved)."""
    uid = 0
    for f in nc.m.functions:
        for bb in f.blocks:
            out = []
            changed = False
            for inst in bb.instructions:
                si = inst.sync_info
                waits = list(si.on_wait) if (si is not None and si.on_wait) else []
                if len(waits) > 1:
                    changed = True
                    for w in waits[:-1]:
                        uid += 1
                        nop = mybir.InstNoOp(name=f"waitsplit_{uid}", ins=[], outs=[])
                        nop.engine = inst.engine
                        nop.sync_info = mybir.SyncInfo(on_update=[], on_wait=[w])
                        out.append(nop)
                    si.on_wait = [waits[-1]]
                out.append(inst)
            if changed:
                bb.instructions = out
    return nc


def _build():
    nc = bass.Bass("TRN2", target_bir_lowering=False, debug=False, num_devices=NCORES)

    keysT = nc.declare_dram_parameter("keysT", [BL, H, S], KDT, isOutput=False)
    UaT = nc.declare_dram_parameter("UaT", [H, H], KDT, isOutput=False)
    WaT = nc.declare_dram_parameter("WaT", [H, H], BF16, isOutput=False)
    qT = nc.declare_dram_parameter("qT", [H, BL], BF16, isOutput=False)
    bsumT = nc.declare_dram_parameter("bsumT", [P, HJ], F32, isOutput=False)
    vaT = nc.declare_dram_parameter("vaT", [P, OC], KDT, isOutput=False)
    out = nc.declare_dram_parameter("out", [BL, H + S], F32, isOutput=True)

    with TileContext(nc) as tc:
        with (
            tc.tile_pool(name="weights", bufs=1) as wpool,
            tc.tile_pool(name="keys", bufs=2) as kpool,
            tc.tile_pool(name="work", bufs=3) as wkpool,
            tc.tile_pool(name="small", bufs=1) as smpool,
            tc.tile_pool(name="psum", bufs=4, space="PSUM") as ppool,
        ):
            # ---- PE warmup: matmuls with no DMA deps, issued from t=0 so the
            # HAM clock-gate is released before real work arrives ----
            wuptile = smpool.tile([P, SBW], BF16, tag="wup")
            nc.gpsimd.memset(wuptile[:], 0.0)
            ones = smpool.tile([1, P], F32, tag="ones")
            nc.gpsimd.memset(ones[:], 1.0)
            for i in range(N_WARMUP):
                pwu = ppool.tile([P, SBW], F32, tag="e", name=f"pwu_{i}")
                nc.tensor.matmul(pwu[:], wuptile[:, 0:P], wuptile[:],
                                 start=True, stop=True)

            # ---- weights/constants; DMA issue order = priority order ----
            wa_sb = wpool.tile([P, HJ * H], BF16)
            for hj in range(HJ):
                nc.sync.dma_start(wa_sb[:, hj * H:(hj + 1) * H],
                                  WaT[hj * P:(hj + 1) * P, :])
            qt_sb = smpool.tile([P, HJ * BL], BF16)    # [h | hj*BL + b]
            for hj in range(HJ):
                nc.sync.dma_start(qt_sb[:, hj * BL:(hj + 1) * BL],
                                  qT[hj * P:(hj + 1) * P, :])
            bs_sb = smpool.tile([P, HJ], F32)
            nc.sync.dma_start(bs_sb[:], bsumT[:, :])
            va_sb = smpool.tile([P, OC], KDT)
            nc.sync.dma_start(va_sb[:], vaT[:, :])

            ua_sb = wpool.tile([P, HJ * H], KDT)       # [h | hj*H + o]
            kt0 = kpool.tile([P, HJ * S], KDT, tag="kT", name="kt_0")
            for hj in range(HJ):
                nc.sync.dma_start(ua_sb[:, hj * H:(hj + 1) * H],
                                  UaT[hj * P:(hj + 1) * P, :])
                nc.sync.dma_start(kt0[:, hj * S:(hj + 1) * S],
                                  keysT[0, hj * P:(hj + 1) * P, :])

            qbT = smpool.tile([P, OC * BL], F32)       # [o | oc*BL + b]
            ctxT = smpool.tile([P, BL * HJ], F32)      # [h | b*HJ + hj]

            # ---- q_proj (transposed): qbT[o, b] = (Wa @ q)[o] + Wa_b + Ua_b ----
            for oc in range(OC):
                pq = ppool.tile([P, BL], F32, tag="sc", name=f"pq_{oc}")
                for hj in range(HJ):
                    nc.tensor.matmul(
                        pq[:],
                        wa_sb[:, hj * H + oc * P: hj * H + (oc + 1) * P],
                        qt_sb[:, hj * BL:(hj + 1) * BL],
                        start=(hj == 0), stop=(hj == HJ - 1),
                    )
                nc.vector.tensor_scalar_add(
                    qbT[:, oc * BL:(oc + 1) * BL], pq[:], bs_sb[:, oc:oc + 1])

            for b in range(BL):
                if b == 0:
                    kt = kt0
                else:
                    kt = kpool.tile([P, HJ * S], KDT, tag="kT", name=f"kt_{b}")
                    for hj in range(HJ):
                        nc.sync.dma_start(kt[:, hj * S:(hj + 1) * S],
                                          keysT[b, hj * P:(hj + 1) * P, :])

                # ---- eT = tanh(Ua@keysT + qb), all (oc, sb) tiles kept ----
                ets = {}
                for oc in range(OC):
                    pe = [ppool.tile([P, SBW], F32, tag="e", name=f"pe_{b}_{oc}_{sb}")
                          for sb in range(SB)]
                    for hj in range(HJ):
                        lhs = ua_sb[:, hj * H + oc * P: hj * H + (oc + 1) * P]
                        for sb in range(SB):
                            nc.tensor.matmul(
                                pe[sb][:], lhs,
                                kt[:, hj * S + sb * SBW: hj * S + sb * SBW + SBW],
                                start=(hj == 0), stop=(hj == HJ - 1),
                            )
                    for sb in range(SB):
                        et = wkpool.tile([P, SBW], KDT, tag="eT", bufs=36,
                                         name=f"et_{b}_{oc}_{sb}")
                        nc.scalar.activation(
                            et[:], pe[sb][:], TANH,
                            bias=qbT[:, oc * BL + b: oc * BL + b + 1], scale=1.0)
                        ets[(oc, sb)] = et

                # ---- scores: dense matmul run, no ACT->PE coupling ----
                psc = [ppool.tile([1, SBW], F32, tag="sc", name=f"psc_{b}_{sb}")
                       for sb in range(SB)]
                for sb in range(SB):
                    for oc in range(OC):
                        nc.tensor.matmul(
                            psc[sb][:], va_sb[:, oc:oc + 1], ets[(oc, sb)][:],
                            start=(oc == 0), stop=(oc == OC - 1),
                            skip_group_check=True,
                        )
                scores = smpool.tile([1, S], F32, tag="scores", bufs=2,
                                     name=f"scores_{b}")
                for sb in range(SB):
                    nc.scalar.copy(scores[0:1, sb * SBW:(sb + 1) * SBW],
                                   psc[sb][:])

                # ---- softmax over [1, S] ----
                mx = smpool.tile([1, 1], F32, tag="mx", bufs=2, name=f"mx_{b}")
                nc.vector.reduce_max(mx[:], scores[0:1, :], axis=AX_X)
                nmx = smpool.tile([1, 1], F32, tag="nmx", bufs=2, name=f"nmx_{b}")
                nc.scalar.mul(nmx[:], mx[:], -1.0)
                zz = smpool.tile([1, 1], F32, tag="zz", bufs=2, name=f"zz_{b}")
                expv = smpool.tile([1, S], F32, tag="expv", bufs=1,
                                   name=f"expv_{b}")
                nc.scalar.activation(expv[0:1, :], scores[0:1, :], EXP,
                                     bias=nmx[:], scale=1.0, accum_out=zz[:])
                rz = smpool.tile([1, 1], F32, tag="rz", bufs=2, name=f"rz_{b}")
                nc.vector.reciprocal(rz[:], zz[:])
                wrow = smpool.tile([1, S], F32, tag="wrow", bufs=1,
                                   name=f"wrow_{b}")
                nc.vector.tensor_scalar_mul(wrow[0:1, :], expv[0:1, :], rz[:])
                nc.sync.dma_start(out[b, H:H + S], wrow[0:1, :])

                # ---- replicate weights across partitions: rank-1 ones-matmul
                # (wr[p, s] = ones[p] * w[s]) + ACT copy to bf16 ----
                wr = wkpool.tile([P, S], KDT, tag="wrep", bufs=2, name=f"wr_{b}")
                for sb in range(SB):
                    pwr = ppool.tile([P, SBW], F32, tag="sc", name=f"pwr_{b}_{sb}")
                    nc.tensor.matmul(pwr[:], ones[:],
                                     wrow[0:1, sb * SBW:(sb + 1) * SBW],
                                     start=True, stop=True)
                    nc.scalar.copy(wr[:, sb * SBW:(sb + 1) * SBW], pwr[:])

                # ---- context: ctxT[h] = sum_s keysT[h, s] * w[s] (bf16 DVE) ----
                for hj in range(HJ):
                    pr = wkpool.tile([P, S], KDT, tag="prod", bufs=2,
                                     name=f"pr_{b}_{hj}")
                    nc.vector.tensor_tensor(out=pr[:], in0=kt[:, hj * S:(hj + 1) * S],
                                            in1=wr[:], op=MULT)
                    nc.vector.reduce_sum(ctxT[:, b * HJ + hj: b * HJ + hj + 1],
                                         pr[:], axis=AX_X)
                nc.sync.dma_start(out[b, 0:H].rearrange("(hj p) -> p hj", p=P),
                                  ctxT[:, b * HJ:(b + 1) * HJ])

    _split_multi_waits(nc)
    return nc


_NC_CACHE = {}


def _get_nc():
    if "nc" not in _NC_CACHE:
        _NC_CACHE["nc"] = _build()
    return _NC_CACHE["nc"]


LAST_RESULTS = {}


def kernel(**inputs):
    query = np.asarray(inputs["query"], np.float32)    # [B, 1, H]
    keys = np.asarray(inputs["keys"], np.float32)      # [B, S, H]
    Wa_w = np.asarray(inputs["Wa_w"], np.float32)      # [H, H]
    Wa_b = np.asarray(inputs["Wa_b"], np.float32)      # [H]
    Ua_w = np.asarray(inputs["Ua_w"], np.float32)      # [H, H]
    Ua_b = np.asarray(inputs["Ua_b"], np.float32)      # [H]
    Va_w = np.asarray(inputs["Va_w"], np.float32)      # [1, H]
    # Va_b shifts every score equally; softmax is shift-invariant and scores
    # are not returned, so it is dropped.

    keysT = np.empty((B, H, S), dtype=KDT_NP)
    for b in range(B):
        keysT[b] = keys[b].T.astype(KDT_NP)
    UaT = np.ascontiguousarray(Ua_w.T).astype(KDT_NP)
    WaT = np.ascontiguousarray(Wa_w.T).astype(KDT_NP)
    bsum = Wa_b + Ua_b
    bsumT = np.ascontiguousarray(bsum.reshape(HJ, P).T)
    vaT = np.ascontiguousarray(Va_w[0].reshape(OC, P).T).astype(KDT_NP)
    queryT = np.ascontiguousarray(query[:, 0, :].T).astype(KDT_NP)  # [H, B]

    in_maps = []
    for c in range(NCORES):
        bsl = slice(c * BL, (c + 1) * BL)
        in_maps.append({
            "keysT": keysT[bsl],
            "UaT": UaT,
            "WaT": WaT,
            "qT": np.ascontiguousarray(queryT[:, bsl]),
            "bsumT": bsumT,
            "vaT": vaT,
        })

    nc = _get_nc()
    trace = bool(int(os.environ.get("KERNEL_TRACE", "0")))
    res = run_bass_kernel_spmd(nc, in_maps, core_ids=list(range(NCORES)),
                               trace=trace)
    LAST_RESULTS["exec_time_ns"] = res.exec_time_ns
    LAST_RESULTS["bass_results"] = res

    full = np.concatenate([np.asarray(res.results[c]["out"]) for c in range(NCORES)],
                          axis=0)                      # [B, H+S]
    context = np.ascontiguousarray(full[:, :H].reshape(B, 1, H), dtype=np.float32)
    weights = np.ascontiguousarray(full[:, H:].reshape(B, 1, S), dtype=np.float32)
    return (context, weights)


# revision 12
# speedup vs baseline: 1.1579x; 1.0480x over previous
"""Additive-attention layer (Bahdanau-style) on 8 TRN2 NeuronCores.

Reference computation (per batch b):
    q_proj = query @ Wa_w.T + Wa_b                      # [1, H]
    k_proj = keys  @ Ua_w.T + Ua_b                      # [S, H]
    e      = tanh(q_proj + k_proj)                      # [S, H]
    scores = e @ Va_w.T (+ Va_b)                        # [S]  (Va_b dropped:
                                                        #  softmax shift-invariant)
    weights = softmax(scores)                           # [S]
    context = weights @ keys                            # [H]
    returns (context [B,1,H], weights [B,1,S])

Sharding: data-parallel over batch B=32 -> 4 batches per core; the small
Wa/Ua/Va weights are replicated. No collectives; the host concatenates
per-core outputs.

Device-side layout:
  - keys ship pre-transposed per batch as keysT [H, S] (bf16): TensorEngine
    contracts over H with no on-device transposes, and the context reduction
    over S runs on the VectorEngine against the resident keysT.
  - Ua_w/Wa_w ship transposed ([h, o]) to slice directly into matmul lhsT.
  - e is produced transposed (eT [o, s]); the scores reduction over o is a
    TensorE matmul against Va, done as a dense run per batch (decoupled from
    the tanh pipeline), and softmax lands in [1, S] on one partition.
  - q_proj folds into the tanh as a per-partition activation bias.
  - weights are replicated across partitions for the context reduction via a
    TensorE ones-matmul (rank-1 broadcast), avoiding slow gather DMAs.

bf16 inputs for the big matmuls, fp32 PSUM accumulation. Measured rel err vs
the fp32 reference ~2.4e-3.
"""

import os
import numpy as np
import ml_dtypes

import concourse.bass as bass
import concourse.mybir as mybir
from concourse.tile import TileContext
from concourse.bass_utils import run_bass_kernel_spmd

B, S, H = 32, 2048, 1024
NCORES = 8
BL = B // NCORES          # batches per core = 4
P = 128                   # partitions
HJ = H // P               # h-chunks = 8
OC = H // P               # o-chunks = 8
SBW = 512                 # s-block width (PSUM bank = 512 fp32)
SB = S // SBW             # s-blocks = 4

F32 = mybir.dt.float32
BF16 = mybir.dt.bfloat16
KDT = BF16
KDT_NP = ml_dtypes.bfloat16

TANH = mybir.ActivationFunctionType.Tanh
EXP = mybir.ActivationFunctionType.Exp
MULT = mybir.AluOpType.mult
AX_X = mybir.AxisListType.X

N_WARMUP = 36             # PE warmup matmuls (cover the HAM cold window)


def _split_multi_waits(nc):
    """This container's walrus rejects >1 sync-wait per instruction. Hoist
    extra waits onto NoOps inserted just before, on the same engine (engines
    run their stream in order, so happens-before edges are preserved)."""
    uid = 0
    for f in nc.m.functions:
        for bb in f.blocks:
            out = []
            changed = False
            for inst in bb.instructions:
                si = inst.sync_info
                waits = list(si.on_wait) if (si is not None and si.on_wait) else []
                if len(waits) > 1:
                    changed = True
                    for w in waits[:-1]:
                        uid += 1
                        nop = mybir.InstNoOp(name=f"waitsplit_{uid}", ins=[], outs=[])
                        nop.engine = inst.engine
                        nop.sync_info = mybir.SyncInfo(on_update=[], on_wait=[w])
                        out.append(nop)
                    si.on_wait = [waits[-1]]
                out.append(inst)
            if changed:
                bb.instructions = out
    return nc


def _build():
    nc = bass.Bass("TRN2", target_bir_lowering=False, debug=False, num_devices=NCORES)

    keysT = nc.declare_dram_parameter("keysT", [BL, H, S], KDT, isOutput=False)
    UaT = nc.declare_dram_parameter("UaT", [H, H], KDT, isOutput=False)
    WaT = nc.declare_dram_parameter("WaT", [H, H], BF16, isOutput=False)
    qT = nc.declare_dram_parameter("qT", [H, BL], BF16, isOutput=False)
    bsumT = nc.declare_dram_parameter("bsumT", [P, HJ], F32, isOutput=False)
    # Va replicated across 128 columns: scores matmuls run at M=128 (full
    # array) so the HAM activity monitor keeps the PE clock at 2.4 GHz.
    vaRT = nc.declare_dram_parameter("vaRT", [P, OC * P], KDT, isOutput=False)
    out = nc.declare_dram_parameter("out", [BL, H + S], F32, isOutput=True)

    with TileContext(nc) as tc:
        with (
            tc.tile_pool(name="weights", bufs=1) as wpool,
            tc.tile_pool(name="keys", bufs=2) as kpool,
            tc.tile_pool(name="work", bufs=3) as wkpool,
            tc.tile_pool(name="small", bufs=1) as smpool,
            tc.tile_pool(name="psum", bufs=4, space="PSUM") as ppool,
        ):
            # ---- PE warmup: matmuls with no DMA deps, issued from t=0 so the
            # HAM clock-gate is released before real work arrives ----
            wuptile = smpool.tile([P, SBW], BF16, tag="wup")
            nc.gpsimd.memset(wuptile[:], 0.0)
            ones = smpool.tile([1, P], F32, tag="ones")
            nc.gpsimd.memset(ones[:], 1.0)
            for i in range(N_WARMUP):
                pwu = ppool.tile([P, SBW], F32, tag="e", name=f"pwu_{i}")
                nc.tensor.matmul(pwu[:], wuptile[:, 0:P], wuptile[:],
                                 start=True, stop=True)

            # ---- weights/constants; DMA issue order = priority order ----
            wa_sb = wpool.tile([P, HJ * H], BF16)
            for hj in range(HJ):
                nc.sync.dma_start(wa_sb[:, hj * H:(hj + 1) * H],
                                  WaT[hj * P:(hj + 1) * P, :])
            qt_sb = smpool.tile([P, HJ * BL], BF16)    # [h | hj*BL + b]
            for hj in range(HJ):
                nc.sync.dma_start(qt_sb[:, hj * BL:(hj + 1) * BL],
                                  qT[hj * P:(hj + 1) * P, :])
            bs_sb = smpool.tile([P, HJ], F32)
            nc.sync.dma_start(bs_sb[:], bsumT[:, :])
            va_sb = smpool.tile([P, OC * P], KDT)
            nc.sync.dma_start(va_sb[:], vaRT[:, :])

            ua_sb = wpool.tile([P, HJ * H], KDT)       # [h | hj*H + o]
            kt0 = kpool.tile([P, HJ * S], KDT, tag="kT", name="kt_0")
            for hj in range(HJ):
                nc.sync.dma_start(ua_sb[:, hj * H:(hj + 1) * H],
                                  UaT[hj * P:(hj + 1) * P, :])
                nc.sync.dma_start(kt0[:, hj * S:(hj + 1) * S],
                                  keysT[0, hj * P:(hj + 1) * P, :])

            qbT = smpool.tile([P, OC * BL], F32)       # [o | oc*BL + b]
            ctxT = smpool.tile([P, BL * HJ], F32)      # [h | b*HJ + hj]

            # ---- q_proj (transposed): qbT[o, b] = (Wa @ q)[o] + Wa_b + Ua_b ----
            for oc in range(OC):
                pq = ppool.tile([P, BL], F32, tag="sc", name=f"pq_{oc}")
                for hj in range(HJ):
                    nc.tensor.matmul(
                        pq[:],
                        wa_sb[:, hj * H + oc * P: hj * H + (oc + 1) * P],
                        qt_sb[:, hj * BL:(hj + 1) * BL],
                        start=(hj == 0), stop=(hj == HJ - 1),
                    )
                nc.vector.tensor_scalar_add(
                    qbT[:, oc * BL:(oc + 1) * BL], pq[:], bs_sb[:, oc:oc + 1])

            for b in range(BL):
                if b == 0:
                    kt = kt0
                else:
                    kt = kpool.tile([P, HJ * S], KDT, tag="kT", name=f"kt_{b}")
                    for hj in range(HJ):
                        nc.sync.dma_start(kt[:, hj * S:(hj + 1) * S],
                                          keysT[b, hj * P:(hj + 1) * P, :])

                # ---- eT = tanh(Ua@keysT + qb), all (oc, sb) tiles kept ----
                ets = {}
                for oc in range(OC):
                    pe = [ppool.tile([P, SBW], F32, tag="e", name=f"pe_{b}_{oc}_{sb}")
                          for sb in range(SB)]
                    for hj in range(HJ):
                        lhs = ua_sb[:, hj * H + oc * P: hj * H + (oc + 1) * P]
                        for sb in range(SB):
                            nc.tensor.matmul(
                                pe[sb][:], lhs,
                                kt[:, hj * S + sb * SBW: hj * S + sb * SBW + SBW],
                                start=(hj == 0), stop=(hj == HJ - 1),
                            )
                    for sb in range(SB):
                        et = wkpool.tile([P, SBW], KDT, tag="eT", bufs=36,
                                         name=f"et_{b}_{oc}_{sb}")
                        nc.scalar.activation(
                            et[:], pe[sb][:], TANH,
                            bias=qbT[:, oc * BL + b: oc * BL + b + 1], scale=1.0)
                        ets[(oc, sb)] = et

                # ---- scores: dense matmul run at M=128 (all output rows
                # identical), decoupled from the tanh pipeline ----
                psc = [ppool.tile([P, SBW], F32, tag="sc", name=f"psc_{b}_{sb}")
                       for sb in range(SB)]
                for sb in range(SB):
                    for oc in range(OC):
                        nc.tensor.matmul(
                            psc[sb][:], va_sb[:, oc * P:(oc + 1) * P],
                            ets[(oc, sb)][:],
                            start=(oc == 0), stop=(oc == OC - 1),
                            skip_group_check=True,
                        )
                scores = smpool.tile([1, S], F32, tag="scores", bufs=2,
                                     name=f"scores_{b}")
                for sb in range(SB):
                    nc.scalar.copy(scores[0:1, sb * SBW:(sb + 1) * SBW],
                                   psc[sb][0:1, :])

                # ---- softmax over [1, S] ----
                mx = smpool.tile([1, 1], F32, tag="mx", bufs=2, name=f"mx_{b}")
                nc.vector.reduce_max(mx[:], scores[0:1, :], axis=AX_X)
                nmx = smpool.tile([1, 1], F32, tag="nmx", bufs=2, name=f"nmx_{b}")
                nc.scalar.mul(nmx[:], mx[:], -1.0)
                zz = smpool.tile([1, 1], F32, tag="zz", bufs=2, name=f"zz_{b}")
                expv = smpool.tile([1, S], F32, tag="expv", bufs=1,
                                   name=f"expv_{b}")
                nc.scalar.activation(expv[0:1, :], scores[0:1, :], EXP,
                                     bias=nmx[:], scale=1.0, accum_out=zz[:])
                rz = smpool.tile([1, 1], F32, tag="rz", bufs=2, name=f"rz_{b}")
                nc.vector.reciprocal(rz[:], zz[:])
                wrow = smpool.tile([1, S], F32, tag="wrow", bufs=1,
                                   name=f"wrow_{b}")
                nc.vector.tensor_scalar_mul(wrow[0:1, :], expv[0:1, :], rz[:])
                nc.sync.dma_start(out[b, H:H + S], wrow[0:1, :])

                # ---- replicate weights across partitions: rank-1 ones-matmul
                # (wr[p, s] = ones[p] * w[s]) + ACT copy to bf16 ----
                wr = wkpool.tile([P, S], KDT, tag="wrep", bufs=2, name=f"wr_{b}")
                for sb in range(SB):
                    pwr = ppool.tile([P, SBW], F32, tag="sc", name=f"pwr_{b}_{sb}")
                    nc.tensor.matmul(pwr[:], ones[:],
                                     wrow[0:1, sb * SBW:(sb + 1) * SBW],
                                     start=True, stop=True)
                    nc.scalar.copy(wr[:, sb * SBW:(sb + 1) * SBW], pwr[:])

                # ---- context: ctxT[h] = sum_s keysT[h, s] * w[s], one fused
                # multiply+accumulate pass per h-block; tail blocks offloaded
                # to the otherwise-idle GpSimd engine ----
                for hj in range(HJ):
                    pr = wkpool.tile([P, S], KDT, tag="prod", bufs=4,
                                     name=f"pr_{b}_{hj}")
                    nc.vector.scalar_tensor_tensor(
                        out=pr[:], in0=kt[:, hj * S:(hj + 1) * S], scalar=1.0,
                        in1=wr[:], op0=mybir.AluOpType.mult, op1=MULT,
                        accum_out=ctxT[:, b * HJ + hj: b * HJ + hj + 1])
                nc.sync.dma_start(out[b, 0:H].rearrange("(hj p) -> p hj", p=P),
                                  ctxT[:, b * HJ:(b + 1) * HJ])

    _split_multi_waits(nc)
    return nc


_NC_CACHE = {}


def _get_nc():
    if "nc" not in _NC_CACHE:
        _NC_CACHE["nc"] = _build()
    return _NC_CACHE["nc"]


LAST_RESULTS = {}


def kernel(**inputs):
    query = np.asarray(inputs["query"], np.float32)    # [B, 1, H]
    keys = np.asarray(inputs["keys"], np.float32)      # [B, S, H]
    Wa_w = np.asarray(inputs["Wa_w"], np.float32)      # [H, H]
    Wa_b = np.asarray(inputs["Wa_b"], np.float32)      # [H]
    Ua_w = np.asarray(inputs["Ua_w"], np.float32)      # [H, H]
    Ua_b = np.asarray(inputs["Ua_b"], np.float32)      # [H]
    Va_w = np.asarray(inputs["Va_w"], np.float32)      # [1, H]
    # Va_b shifts every score equally; softmax is shift-invariant and scores
    # are not returned, so it is dropped.

    keysT = np.empty((B, H, S), dtype=KDT_NP)
    for b in range(B):
        keysT[b] = keys[b].T.astype(KDT_NP)
    UaT = np.ascontiguousarray(Ua_w.T).astype(KDT_NP)
    WaT = np.ascontiguousarray(Wa_w.T).astype(KDT_NP)
    bsum = Wa_b + Ua_b
    bsumT = np.ascontiguousarray(bsum.reshape(HJ, P).T)
    vaT = np.ascontiguousarray(Va_w[0].reshape(OC, P).T).astype(KDT_NP)
    vaRT = np.ascontiguousarray(np.repeat(vaT, P, axis=1))  # [P, OC*P]
    queryT = np.ascontiguousarray(query[:, 0, :].T).astype(KDT_NP)  # [H, B]

    in_maps = []
    for c in range(NCORES):
        bsl = slice(c * BL, (c + 1) * BL)
        in_maps.append({
            "keysT": keysT[bsl],
            "UaT": UaT,
            "WaT": WaT,
            "qT": np.ascontiguousarray(queryT[:, bsl]),
            "bsumT": bsumT,
            "vaRT": vaRT,
        })

    nc = _get_nc()
    trace = bool(int(os.environ.get("KERNEL_TRACE", "0")))
    res = run_bass_kernel_spmd(nc, in_maps, core_ids=list(range(NCORES)),
                               trace=trace)
    LAST_RESULTS["exec_time_ns"] = res.exec_time_ns
    LAST_RESULTS["bass_results"] = res

    full = np.concatenate([np.asarray(res.results[c]["out"]) for c in range(NCORES)],
                          axis=0)                      # [B, H+S]
    context = np.ascontiguousarray(full[:, :H].reshape(B, 1, H), dtype=np.float32)
    weights = np.ascontiguousarray(full[:, H:].reshape(B, 1, S), dtype=np.float32)
    return (context, weights)
